# revision 3
# baseline (speedup 1.0000x reference)
"""Trainium2 Bass kernel for nn_Convolution_1176821039998.

Equivariant (e3nn-style) 3D convolution, kernel 5x5x5, 64->64 channels, on a
[1,64,56,56,56] fp32 volume, plus a per-irrep self-connection on the cropped
volume.  Strategy:

Host side (tiny, fp32):
  - Build the dense conv kernel K[o,i,dz,dy,dx] from the TP weight exactly as
    the reference does, and fold the self-connection into the center tap
    (the self-connection is a 64x64 block-structured linear map applied at the
    output voxel = center-tap position).
  - Shard the volume along z across 8 cores: core c computes output planes
    z0..z0+6 (z0 = 7c, clamped for the last core) from input planes
    z0..z0+10 (halo of 4 = 2*(size//2) handled by overlapping the shards, so
    no device-to-device halo exchange is needed).
  - Pack per-core inputs as a dual z-shifted fp16 slab on 128 SBUF
    partitions: partitions 0..63 hold channels 0..63 of planes z0..z0+10,
    partitions 64..127 hold the same channels shifted by one plane
    (z0+1..z0+11).  A single matmul with K=128 then contracts TWO kernel
    z-taps at once (64 channels x 2 taps).

Device side (per core, identical SPMD program):
  - 7 output planes x 6 line-chunks (9/9/9/9/9/7 lines of 52) = 42 PSUM
    tiles [64, L*52] fp32.
  - Per tile, 75 accumulating matmuls: 25 (dy,dx) x 3 z-groups
    (dz 0&1 | dz 2&3 | dz 4 with zero top half), lhsT = [128,64] fp16
    weights, rhs = in-SBUF shifted window [128, L, 52].
  - Evict PSUM via VectorE copy, DMA out [64, 7, 52, 52] fp32.

Numerics: inputs/weights in fp16 (products exact in fp32, PSUM accumulates
fp32); measured max rel err vs fp32 reference ~4e-4.
"""

import os
import numpy as np

import concourse.bass as bass
import concourse.mybir as mybir
import concourse.tile as tile
from concourse import bacc
from concourse.bass_utils import run_bass_kernel_spmd

# ---------------------------------------------------------------- constants
SIZE = 5
MUL = 16
CROP = SIZE // 2
PW0 = np.float32((1.0 / 32.0) ** 0.5)
PW1 = np.float32((3.0 / 32.0) ** 0.5)
INV_SQRT3 = np.float32(3.0 ** -0.5)

N_CORES = 8
Z0S = [0, 7, 14, 21, 28, 35, 42, 45]   # per-core first output plane
D_OUT = 7                              # output planes per core
D_IN = 11                              # input planes per core (halo 4)
S = 56                                 # input spatial size
SO = 52                                # output spatial size
CHUNKS = [(0, 9), (9, 9), (18, 9), (27, 9), (36, 9), (45, 7)]  # (y0, lines)
# matmul group order: (dy, dx, zg); zg 0 -> dz (0,1), 1 -> (2,3), 2 -> (4,-)
GROUPS = [(dy, dx, zg) for dy in range(5) for dx in range(5) for zg in range(3)]
NG = len(GROUPS)  # 75


# ------------------------------------------------------- host-side weights
def _lattice_consts():
    r = np.linspace(-1.0, 1.0, SIZE, dtype=np.float32)
    lat = np.stack(np.meshgrid(r, r, r, indexing="ij"), axis=-1)
    d = np.linalg.norm(lat.astype(np.float64), axis=-1).astype(np.float32)
    values = np.linspace(0.0, 1.0, SIZE, dtype=np.float32)
    step = values[1] - values[0]
    diff = (d[..., None] - values) / step

    def sus(t):
        return np.where(t > 0, np.exp(-1.0 / np.where(t > 0, t, 1.0)), 0.0).astype(
            np.float32
        )

    emb = np.float32(1.14136) * np.float32(np.e ** 2) * sus(diff + 1.0) * sus(1.0 - diff)
    n = lat / np.maximum(d, 1e-12)[..., None]
    sh0 = np.ones_like(d)
    sh1 = np.float32(3.0 ** 0.5) * n
    return emb.astype(np.float32), sh0, sh1.astype(np.float32)


def _make_kernel(weight):
    """[5,1024] -> conv kernel [out=64, in=64, 5,5,5] fp32 (mirrors reference)."""
    emb, sh0, sh1 = _lattice_consts()
    w = emb @ weight
    Ssp = w.shape[:3]
    blk = MUL * MUL
    w1, w2, w3, w4 = [
        w[..., i * blk : (i + 1) * blk].reshape(*Ssp, MUL, MUL) for i in range(4)
    ]
    k_ss = PW0 * w1 * sh0[..., None, None]
    k_sv = PW1 * INV_SQRT3 * np.einsum("...uw,...k->...uwk", w2, sh1)
    k_vs = PW0 * INV_SQRT3 * np.einsum("...uw,...i->...uiw", w4, sh1)
    eye3 = np.eye(3, dtype=w.dtype)
    k_vv = (
        PW1
        * INV_SQRT3
        * (w3 * sh0[..., None, None])[..., :, None, :, None]
        * eye3[None, None, None, None, :, None, :]
    )
    top = np.concatenate([k_ss, k_sv.reshape(*Ssp, MUL, 3 * MUL)], axis=-1)
    bot = np.concatenate(
        [k_vs.reshape(*Ssp, 3 * MUL, MUL), k_vv.reshape(*Ssp, 3 * MUL, 3 * MUL)],
        axis=-1,
    )
    kernel = np.concatenate([top, bot], axis=-2)  # [5,5,5,in,out]
    return np.ascontiguousarray(np.transpose(kernel, (4, 3, 0, 1, 2)))


def _fold_self_connection(K, w_sc0, w_sc1):
    """Add the cropped e3nn Linear self-connection into the center tap."""
    inv = np.float32(1.0 / MUL ** 0.5)
    sc = np.zeros((64, 64), np.float32)
    sc[:MUL, :MUL] = w_sc0.T * inv  # sc[out w, in u] = w_sc0[u, w]
    for wo in range(MUL):
        for u in range(MUL):
            for k in range(3):
                sc[MUL + 3 * wo + k, MUL + 3 * u + k] += w_sc1[u, wo] * inv
    K = K.copy()
    K[:, :, CROP, CROP, CROP] += sc
    return K


def _pack_weights(K, dtype=np.float16):
    """[64,64,5,5,5] -> lhsT tiles [128, NG, 64] in GROUPS order."""
    wk = np.zeros((128, NG, 64), np.float32)
    for g, (dy, dx, zg) in enumerate(GROUPS):
        dz_a = (0, 2, 4)[zg]
        wk[:64, g, :] = K[:, :, dz_a, dy, dx].T  # [in, out]
        if zg < 2:
            wk[64:, g, :] = K[:, :, dz_a + 1, dy, dx].T
    return np.ascontiguousarray(wk.astype(dtype))


def _pack_x(x, dtype=np.float16):
    """x [1,64,56,56,56] -> per-core dual z-shifted slabs [128, 11, 56, 56]."""
    slabs = []
    for z0 in Z0S:
        xa = x[0, :, z0 : z0 + D_IN]
        xb = np.zeros((64, D_IN, S, S), np.float32)
        avail = min(S - (z0 + 1), D_IN)
        xb[:, :avail] = x[0, :, z0 + 1 : z0 + 1 + avail]
        slabs.append(
            np.ascontiguousarray(np.concatenate([xa, xb], axis=0).astype(dtype))
        )
    return slabs


# ------------------------------------------------------- device program
def build_nc(d_out=D_OUT, repeat=1):
    fp16 = mybir.dt.float16
    fp32 = mybir.dt.float32
    nc = bacc.Bacc("TRN2", target_bir_lowering=False, debug=False,
                   num_devices=N_CORES)
    x_d = nc.dram_tensor("x", [128, D_IN, S, S], fp16, kind="ExternalInput").ap()
    w_d = nc.dram_tensor("w", [128, NG, 64], fp16, kind="ExternalInput").ap()
    o_d = nc.dram_tensor("out", [64, d_out, SO, SO], fp32,
                         kind="ExternalOutput").ap()

    with tile.TileContext(nc) as tc:
        with (
            tc.tile_pool(name="const", bufs=1) as cpool,
            tc.tile_pool(name="outp", bufs=3) as opool,
            tc.tile_pool(name="psum", bufs=4, space="PSUM") as ppool,
        ):
            xt = cpool.tile([128, D_IN, S, S], fp16)
            wt = cpool.tile([128, NG, 64], fp16)
            nc.sync.dma_start(wt[:], w_d)
            for j in range(D_IN):
                nc.sync.dma_start(xt[:, j], x_d[:, j])

            for _ in range(repeat):
                for z in range(d_out):
                    for ys, L in CHUNKS:
                        ps = ppool.tile([64, 9, SO], fp32)
                        for g, (dy, dx, zg) in enumerate(GROUPS):
                            dz_a = (0, 2, 4)[zg]
                            rhs = xt[:, z + dz_a, ys + dy : ys + dy + L,
                                     dx : dx + SO]
                            nc.tensor.matmul(
                                ps[:, :L, :], wt[:, g], rhs,
                                start=(g == 0), stop=(g == NG - 1),
                            )
                        ot = opool.tile([64, 9, SO], fp32)
                        nc.vector.tensor_copy(ot[:, :L], ps[:, :L])
                        nc.sync.dma_start(o_d[:, z, ys : ys + L, :], ot[:, :L])
    nc.compile()
    return nc


# ------------------------------------------------------------ entry point
LAST_RESULTS = None
LAST_NC = None
LAST_INMAPS = None


def kernel(x, weight, w_sc0, w_sc1):
    global LAST_RESULTS, LAST_NC, LAST_INMAPS
    x = np.asarray(x, dtype=np.float32)
    K = _fold_self_connection(
        _make_kernel(np.asarray(weight, dtype=np.float32)),
        np.asarray(w_sc0, dtype=np.float32),
        np.asarray(w_sc1, dtype=np.float32),
    )
    wk = _pack_weights(K)
    slabs = _pack_x(x)

    repeat = int(os.environ.get("KERNEL_REPEAT", "1"))
    nc = build_nc(repeat=repeat)
    in_maps = [{"x": slabs[c], "w": wk} for c in range(N_CORES)]
    res = run_bass_kernel_spmd(nc, in_maps, core_ids=list(range(N_CORES)))
    LAST_RESULTS, LAST_NC, LAST_INMAPS = res, nc, in_maps

    full = np.zeros((1, 64, SO, SO, SO), np.float32)
    for c, z0 in enumerate(Z0S):
        full[0, :, z0 : z0 + D_OUT] = res.results[c]["out"]
    return full


# revision 13
# speedup vs baseline: 1342.0234x; 1342.0234x over previous
"""Trainium2 Bass kernel for nn_Convolution_1176821039998.

Equivariant (e3nn-style) 3D convolution, kernel 5x5x5, 64->64 channels, on a
[1,64,56,56,56] fp32 volume, plus a per-irrep self-connection on the cropped
volume.  Strategy:

Host side (tiny, fp32):
  - Build the dense conv kernel K[o,i,dz,dy,dx] from the TP weight exactly as
    the reference does, and fold the self-connection into the center tap.
  - Perfectly balanced z-shard across 8 cores with NO redundant compute:
    core c computes 6 "main" output planes 6c..6c+5 (planes 0..47) plus a
    13-line y-block of one of the remaining 2 plane-pairs (planes 48..51,
    block chosen by core index).  The partial block's input sub-volume is
    packed by the host into a small side tensor at FIXED local coordinates,
    so all cores run the identical SPMD program (3.25 plane-pairs each).
  - The kernel halo (4 planes) is handled by overlapping shards; no
    device-to-device exchange.

Device side (pair-packed matmuls using the full 128x128 PE array):
  - K = 128: 64 input channels x 2 adjacent input z-planes (dual z-shifted
    SBUF copies, built with two DMAs from one DRAM slab).
  - M = 128: 64 output channels x 2 adjacent OUTPUT z-planes.  A stream
    reading input plane z+s serves output plane z (taps dz=s low copy,
    s+1 high) and z+1 (dz=s-1 low, s high); streams s in {0,2,4} cover all
    5 z-taps of both planes -> 75 accumulating matmuls per plane-PAIR
    (25 (dy,dx) x 3), 2x fewer streamed columns than one-plane-at-a-time.
  - PSUM tile [128, L*52] fp32 per line-chunk; evict via VectorE, DMA out.

Numerics: inputs/weights fp16 (products exact in fp32, PSUM accumulates
fp32); measured max rel err vs fp32 reference ~4.5e-4.
"""

import os
import numpy as np

import concourse.bass as bass
import concourse.mybir as mybir
import concourse.tile as tile
from concourse import bacc
from concourse.bass_utils import run_bass_kernel_spmd

# ---------------------------------------------------------------- constants
SIZE = 5
MUL = 16
CROP = SIZE // 2
PW0 = np.float32((1.0 / 32.0) ** 0.5)
PW1 = np.float32((3.0 / 32.0) ** 0.5)
INV_SQRT3 = np.float32(3.0 ** -0.5)

N_CORES = 8
S = 56                                 # input spatial size
SO = 52                                # output spatial size
# main shard: 3 pairs = 6 output planes at z0 = 6c (covers 0..47)
N_PAIRS = 3
D_OUT = 6
D_DRAM = 10                            # main DRAM slab planes (6c..6c+9)
D_SB = 9                               # planes per SBUF copy (lo 0..8, hi 1..9)
# partial shard: planes 48..51 split into 8 (pair, 13-line block) quarters
P2_BASE = 48
P2_LINES = 13
P2_IN_LINES = P2_LINES + 4             # 17
P2_DRAM = 6                            # partial DRAM planes (P..P+5)
P2_SB = 5                              # partial SBUF planes per copy
CHUNKS = [(0, 9), (9, 9), (18, 9), (27, 9), (36, 9), (45, 7)]  # (y0, lines)
CHUNKS2 = [(0, 7), (7, 6)]             # partial 13-line block
# matmul group order: (dy, dx, s) with stream plane offset s
S_STREAMS = (0, 2, 4)
GROUPS = [(dy, dx, s) for dy in range(5) for dx in range(5) for s in S_STREAMS]
NG = len(GROUPS)  # 75


def _core_assign(c):
    """(main z0, partial pair base, partial y0) for core c."""
    return 6 * c, P2_BASE + 2 * (c // 4), P2_LINES * (c % 4)


# ------------------------------------------------------- host-side weights
def _lattice_consts():
    r = np.linspace(-1.0, 1.0, SIZE, dtype=np.float32)
    lat = np.stack(np.meshgrid(r, r, r, indexing="ij"), axis=-1)
    d = np.linalg.norm(lat.astype(np.float64), axis=-1).astype(np.float32)
    values = np.linspace(0.0, 1.0, SIZE, dtype=np.float32)
    step = values[1] - values[0]
    diff = (d[..., None] - values) / step

    def sus(t):
        return np.where(t > 0, np.exp(-1.0 / np.where(t > 0, t, 1.0)), 0.0).astype(
            np.float32
        )

    emb = np.float32(1.14136) * np.float32(np.e ** 2) * sus(diff + 1.0) * sus(1.0 - diff)
    n = lat / np.maximum(d, 1e-12)[..., None]
    sh0 = np.ones_like(d)
    sh1 = np.float32(3.0 ** 0.5) * n
    return emb.astype(np.float32), sh0, sh1.astype(np.float32)


def _make_kernel(weight):
    """[5,1024] -> conv kernel [out=64, in=64, 5,5,5] fp32 (mirrors reference)."""
    emb, sh0, sh1 = _lattice_consts()
    w = emb @ weight
    Ssp = w.shape[:3]
    blk = MUL * MUL
    w1, w2, w3, w4 = [
        w[..., i * blk : (i + 1) * blk].reshape(*Ssp, MUL, MUL) for i in range(4)
    ]
    k_ss = PW0 * w1 * sh0[..., None, None]
    k_sv = PW1 * INV_SQRT3 * np.einsum("...uw,...k->...uwk", w2, sh1)
    k_vs = PW0 * INV_SQRT3 * np.einsum("...uw,...i->...uiw", w4, sh1)
    eye3 = np.eye(3, dtype=w.dtype)
    k_vv = (
        PW1
        * INV_SQRT3
        * (w3 * sh0[..., None, None])[..., :, None, :, None]
        * eye3[None, None, None, None, :, None, :]
    )
    top = np.concatenate([k_ss, k_sv.reshape(*Ssp, MUL, 3 * MUL)], axis=-1)
    bot = np.concatenate(
        [k_vs.reshape(*Ssp, 3 * MUL, MUL), k_vv.reshape(*Ssp, 3 * MUL, 3 * MUL)],
        axis=-1,
    )
    kernel = np.concatenate([top, bot], axis=-2)  # [5,5,5,in,out]
    return np.ascontiguousarray(np.transpose(kernel, (4, 3, 0, 1, 2)))


def _fold_self_connection(K, w_sc0, w_sc1):
    """Add the cropped e3nn Linear self-connection into the center tap."""
    inv = np.float32(1.0 / MUL ** 0.5)
    sc = np.zeros((64, 64), np.float32)
    sc[:MUL, :MUL] = w_sc0.T * inv  # sc[out w, in u] = w_sc0[u, w]
    for wo in range(MUL):
        for u in range(MUL):
            for k in range(3):
                sc[MUL + 3 * wo + k, MUL + 3 * u + k] += w_sc1[u, wo] * inv
    K = K.copy()
    K[:, :, CROP, CROP, CROP] += sc
    return K


def _pack_weights(K, dtype=np.float16):
    """[64,64,5,5,5] -> lhsT tiles [128, NG, 128] in GROUPS order.

    lhsT rows: 64 in-channels x {low copy (plane z+s), high copy (z+s+1)}.
    lhsT cols: 64 out-channels x {out plane z, out plane z+1}.
    Block (row half r, col half m) holds tap dz = s + r - m (zero if outside
    0..4)."""
    wk = np.zeros((128, NG, 128), np.float32)
    for g, (dy, dx, s) in enumerate(GROUPS):
        for r in range(2):
            for m in range(2):
                dz = s + r - m
                if 0 <= dz < 5:
                    wk[64 * r : 64 * r + 64, g, 64 * m : 64 * m + 64] = K[
                        :, :, dz, dy, dx
                    ].T
    return np.ascontiguousarray(wk.astype(dtype))


def _pack_x(x, dtype=np.float16):
    """x [1,64,56,56,56] -> per-core (main slab [64,10,56,56],
    partial slab [64,6,17,56])."""
    slabs = []
    for c in range(N_CORES):
        z0, p2, y2 = _core_assign(c)
        xa = np.ascontiguousarray(x[0, :, z0 : z0 + D_DRAM].astype(dtype))
        xp = np.ascontiguousarray(
            x[0, :, p2 : p2 + P2_DRAM, y2 : y2 + P2_IN_LINES].astype(dtype)
        )
        slabs.append((xa, xp))
    return slabs


# ------------------------------------------------------- device program
def build_nc(n_pairs=N_PAIRS, partial=True, repeat=1):
    fp16 = mybir.dt.float16
    fp32 = mybir.dt.float32
    nc = bacc.Bacc("TRN2", target_bir_lowering=False, debug=False,
                   num_devices=N_CORES)
    x_d = nc.dram_tensor("x", [64, D_DRAM, S, S], fp16, kind="ExternalInput").ap()
    x2_d = nc.dram_tensor("x2", [64, P2_DRAM, P2_IN_LINES, S], fp16,
                          kind="ExternalInput").ap()
    w_d = nc.dram_tensor("w", [128, NG, 128], fp16, kind="ExternalInput").ap()
    o_d = nc.dram_tensor("out", [64, 2 * n_pairs, SO, SO], fp32,
                         kind="ExternalOutput").ap()
    o2_d = nc.dram_tensor("out2", [64, 2, P2_LINES, SO], fp32,
                          kind="ExternalOutput").ap()

    with tile.TileContext(nc) as tc:
        with (
            tc.tile_pool(name="const", bufs=1) as cpool,
            tc.tile_pool(name="outp", bufs=3) as opool,
            tc.tile_pool(name="psum", bufs=8, space="PSUM") as ppool,
        ):
            xt = cpool.tile([128, D_SB, S, S], fp16)
            xt2 = cpool.tile([128, P2_SB, P2_IN_LINES, S], fp16)
            wt = cpool.tile([128, NG, 128], fp16)
            # DMA order = first-use order: tiny partial slab, then weights in
            # slices (matmul g only gates on its slice), then the main slab.
            # Dual z-shifted SBUF copies are built with two DMAs per plane
            # from the single DRAM slab: partitions 0..63 plane j <- plane j,
            # partitions 64..127 plane j <- plane j+1.
            for j in range(P2_SB):
                nc.sync.dma_start(xt2[:64, j], x2_d[:, j])
                nc.sync.dma_start(xt2[64:, j], x2_d[:, j + 1])
            for i in range(15):
                nc.sync.dma_start(wt[:, 5 * i : 5 * (i + 1)],
                                  w_d[:, 5 * i : 5 * (i + 1)])
            for j in range(D_SB):
                nc.sync.dma_start(xt[:64, j], x_d[:, j])
                nc.sync.dma_start(xt[64:, j], x_d[:, j + 1])

            def do_chunk(src, z, ys, L, dst, zo):
                ps = ppool.tile([128, 9, SO], fp32)
                for g, (dy, dx, s) in enumerate(GROUPS):
                    rhs = src[:, z + s, ys + dy : ys + dy + L, dx : dx + SO]
                    nc.tensor.matmul(ps[:, :L, :], wt[:, g], rhs,
                                     start=(g == 0), stop=(g == NG - 1))
                ot = opool.tile([128, 9, SO], fp32)
                nc.vector.tensor_copy(ot[:, :L], ps[:, :L])
                nc.sync.dma_start(dst[:, zo, ys : ys + L, :], ot[:64, :L])
                nc.sync.dma_start(dst[:, zo + 1, ys : ys + L, :], ot[64:, :L])

            for _ in range(repeat):
                # partial first: its input lands quickly, hiding the main
                # slab's DMA behind ~21us of compute
                if partial:
                    for ys, L in CHUNKS2:
                        do_chunk(xt2, 0, ys, L, o2_d, 0)
                for p in range(n_pairs):
                    for ys, L in CHUNKS:
                        do_chunk(xt, 2 * p, ys, L, o_d, 2 * p)
    nc.compile()
    return nc


# ------------------------------------------------------------ entry point
LAST_RESULTS = None
LAST_NC = None
LAST_INMAPS = None


def kernel(x, weight, w_sc0, w_sc1):
    global LAST_RESULTS, LAST_NC, LAST_INMAPS
    x = np.asarray(x, dtype=np.float32)
    K = _fold_self_connection(
        _make_kernel(np.asarray(weight, dtype=np.float32)),
        np.asarray(w_sc0, dtype=np.float32),
        np.asarray(w_sc1, dtype=np.float32),
    )
    wk = _pack_weights(K)
    slabs = _pack_x(x)

    repeat = int(os.environ.get("KERNEL_REPEAT", "1"))
    nc = build_nc(repeat=repeat)
    in_maps = [{"x": slabs[c][0], "x2": slabs[c][1], "w": wk}
               for c in range(N_CORES)]
    res = run_bass_kernel_spmd(nc, in_maps, core_ids=list(range(N_CORES)))
    LAST_RESULTS, LAST_NC, LAST_INMAPS = res, nc, in_maps

    full = np.zeros((1, 64, SO, SO, SO), np.float32)
    for c in range(N_CORES):
        z0, p2, y2 = _core_assign(c)
        full[0, :, z0 : z0 + D_OUT] = res.results[c]["out"]
        full[0, :, p2 : p2 + 2, y2 : y2 + P2_LINES, :] = res.results[c]["out2"]
    return full


# revision 18
# speedup vs baseline: 1370.5284x; 1.0212x over previous
"""Trainium2 Bass kernel for nn_Convolution_1176821039998.

Equivariant (e3nn-style) 3D convolution, kernel 5x5x5, 64->64 channels, on a
[1,64,56,56,56] fp32 volume, plus a per-irrep self-connection on the cropped
volume.  Strategy:

Host side (tiny, fp32):
  - Build the dense conv kernel K[o,i,dz,dy,dx] from the TP weight exactly as
    the reference does, and fold the self-connection into the center tap.
  - Perfectly balanced z-shard across 8 cores with NO redundant compute:
    core c computes 6 "main" output planes 6c..6c+5 (planes 0..47) plus a
    13-line y-block of one of the remaining 2 plane-pairs (planes 48..51,
    block chosen by core index).  The partial block's input sub-volume is
    packed by the host into a small side tensor at FIXED local coordinates,
    so all cores run the identical SPMD program (3.25 plane-pairs each).
  - The kernel halo (4 planes) is handled by overlapping shards; no
    device-to-device exchange.

Device side (pair-packed matmuls using the full 128x128 PE array):
  - K = 128: 64 input channels x 2 adjacent input z-planes (dual z-shifted
    SBUF copies, built with two DMAs from one DRAM slab).
  - M = 128: 64 output channels x 2 adjacent OUTPUT z-planes.  A stream
    reading input plane z+s serves output plane z (taps dz=s low copy,
    s+1 high) and z+1 (dz=s-1 low, s high); streams s in {0,2,4} cover all
    5 z-taps of both planes -> 75 accumulating matmuls per plane-PAIR
    (25 (dy,dx) x 3), 2x fewer streamed columns than one-plane-at-a-time.
  - PSUM tile [128, L*52] fp32 per line-chunk; evict via VectorE, DMA out.

Numerics: inputs/weights fp16 (products exact in fp32, PSUM accumulates
fp32); measured max rel err vs fp32 reference ~4.5e-4.
"""

import os
import numpy as np

import concourse.bass as bass
import concourse.mybir as mybir
import concourse.tile as tile
from concourse import bacc
from concourse.bass_utils import run_bass_kernel_spmd

# ---------------------------------------------------------------- constants
SIZE = 5
MUL = 16
CROP = SIZE // 2
PW0 = np.float32((1.0 / 32.0) ** 0.5)
PW1 = np.float32((3.0 / 32.0) ** 0.5)
INV_SQRT3 = np.float32(3.0 ** -0.5)

N_CORES = 8
S = 56                                 # input spatial size
SO = 52                                # output spatial size
# main shard: 3 pairs = 6 output planes at z0 = 6c (covers 0..47)
N_PAIRS = 3
D_OUT = 6
D_DRAM = 10                            # main DRAM slab planes (6c..6c+9)
D_SB = 9                               # planes per SBUF copy (lo 0..8, hi 1..9)
# partial shard: planes 48..51 split into 8 (pair, 13-line block) quarters
P2_BASE = 48
P2_LINES = 13
P2_IN_LINES = P2_LINES + 4             # 17
P2_DRAM = 6                            # partial DRAM planes (P..P+5)
P2_SB = 5                              # partial SBUF planes per copy
CHUNKS = [(0, 9), (9, 9), (18, 9), (27, 9), (36, 9), (45, 7)]  # (y0, lines)
CHUNKS2 = [(0, 7), (7, 6)]             # partial 13-line block
# matmul group order: (dy, dx, s) with stream plane offset s
S_STREAMS = (0, 2, 4)
GROUPS = [(dy, dx, s) for dy in range(5) for dx in range(5) for s in S_STREAMS]
NG = len(GROUPS)  # 75


def _core_assign(c):
    """(main z0, partial pair base, partial y0) for core c."""
    return 6 * c, P2_BASE + 2 * (c // 4), P2_LINES * (c % 4)


# ------------------------------------------------------- host-side weights
def _lattice_consts():
    r = np.linspace(-1.0, 1.0, SIZE, dtype=np.float32)
    lat = np.stack(np.meshgrid(r, r, r, indexing="ij"), axis=-1)
    d = np.linalg.norm(lat.astype(np.float64), axis=-1).astype(np.float32)
    values = np.linspace(0.0, 1.0, SIZE, dtype=np.float32)
    step = values[1] - values[0]
    diff = (d[..., None] - values) / step

    def sus(t):
        return np.where(t > 0, np.exp(-1.0 / np.where(t > 0, t, 1.0)), 0.0).astype(
            np.float32
        )

    emb = np.float32(1.14136) * np.float32(np.e ** 2) * sus(diff + 1.0) * sus(1.0 - diff)
    n = lat / np.maximum(d, 1e-12)[..., None]
    sh0 = np.ones_like(d)
    sh1 = np.float32(3.0 ** 0.5) * n
    return emb.astype(np.float32), sh0, sh1.astype(np.float32)


def _make_kernel(weight):
    """[5,1024] -> conv kernel [out=64, in=64, 5,5,5] fp32 (mirrors reference)."""
    emb, sh0, sh1 = _lattice_consts()
    w = emb @ weight
    Ssp = w.shape[:3]
    blk = MUL * MUL
    w1, w2, w3, w4 = [
        w[..., i * blk : (i + 1) * blk].reshape(*Ssp, MUL, MUL) for i in range(4)
    ]
    k_ss = PW0 * w1 * sh0[..., None, None]
    k_sv = PW1 * INV_SQRT3 * np.einsum("...uw,...k->...uwk", w2, sh1)
    k_vs = PW0 * INV_SQRT3 * np.einsum("...uw,...i->...uiw", w4, sh1)
    eye3 = np.eye(3, dtype=w.dtype)
    k_vv = (
        PW1
        * INV_SQRT3
        * (w3 * sh0[..., None, None])[..., :, None, :, None]
        * eye3[None, None, None, None, :, None, :]
    )
    top = np.concatenate([k_ss, k_sv.reshape(*Ssp, MUL, 3 * MUL)], axis=-1)
    bot = np.concatenate(
        [k_vs.reshape(*Ssp, 3 * MUL, MUL), k_vv.reshape(*Ssp, 3 * MUL, 3 * MUL)],
        axis=-1,
    )
    kernel = np.concatenate([top, bot], axis=-2)  # [5,5,5,in,out]
    return np.ascontiguousarray(np.transpose(kernel, (4, 3, 0, 1, 2)))


def _fold_self_connection(K, w_sc0, w_sc1):
    """Add the cropped e3nn Linear self-connection into the center tap."""
    inv = np.float32(1.0 / MUL ** 0.5)
    sc = np.zeros((64, 64), np.float32)
    sc[:MUL, :MUL] = w_sc0.T * inv  # sc[out w, in u] = w_sc0[u, w]
    for wo in range(MUL):
        for u in range(MUL):
            for k in range(3):
                sc[MUL + 3 * wo + k, MUL + 3 * u + k] += w_sc1[u, wo] * inv
    K = K.copy()
    K[:, :, CROP, CROP, CROP] += sc
    return K


def _pack_weights(K, dtype=np.float16):
    """[64,64,5,5,5] -> lhsT tiles [128, NG, 128] in GROUPS order.

    lhsT rows: 64 in-channels x {low copy (plane z+s), high copy (z+s+1)}.
    lhsT cols: 64 out-channels x {out plane z, out plane z+1}.
    Block (row half r, col half m) holds tap dz = s + r - m (zero if outside
    0..4)."""
    wk = np.zeros((128, NG, 128), np.float32)
    for g, (dy, dx, s) in enumerate(GROUPS):
        for r in range(2):
            for m in range(2):
                dz = s + r - m
                if 0 <= dz < 5:
                    wk[64 * r : 64 * r + 64, g, 64 * m : 64 * m + 64] = K[
                        :, :, dz, dy, dx
                    ].T
    return np.ascontiguousarray(wk.astype(dtype))


def _pack_x(x, dtype=np.float16):
    """x [1,64,56,56,56] -> per-core (main slab [64,10,56,56],
    partial slab [64,6,17,56])."""
    slabs = []
    for c in range(N_CORES):
        z0, p2, y2 = _core_assign(c)
        xa = np.ascontiguousarray(x[0, :, z0 : z0 + D_DRAM].astype(dtype))
        xp = np.ascontiguousarray(
            x[0, :, p2 : p2 + P2_DRAM, y2 : y2 + P2_IN_LINES].astype(dtype)
        )
        slabs.append((xa, xp))
    return slabs


# ------------------------------------------------------- device program
def build_nc(n_pairs=N_PAIRS, partial=True, repeat=1):
    fp16 = mybir.dt.float16
    fp32 = mybir.dt.float32
    nc = bacc.Bacc("TRN2", target_bir_lowering=False, debug=False,
                   num_devices=N_CORES)
    x_d = nc.dram_tensor("x", [64, D_DRAM, S, S], fp16, kind="ExternalInput").ap()
    x2_d = nc.dram_tensor("x2", [64, P2_DRAM, P2_IN_LINES, S], fp16,
                          kind="ExternalInput").ap()
    w_d = nc.dram_tensor("w", [128, NG, 128], fp16, kind="ExternalInput").ap()
    # outputs are plane-major so one DMA can write both planes of a pair:
    # SBUF partitions (z c) = plane-half * 64 + channel
    o_d = nc.dram_tensor("out", [2 * n_pairs, 64, SO, SO], fp32,
                         kind="ExternalOutput").ap()
    o2_d = nc.dram_tensor("out2", [2, 64, P2_LINES, SO], fp32,
                          kind="ExternalOutput").ap()

    with tile.TileContext(nc) as tc:
        with (
            tc.tile_pool(name="const", bufs=1) as cpool,
            tc.tile_pool(name="outp", bufs=3) as opool,
            tc.tile_pool(name="psum", bufs=8, space="PSUM") as ppool,
        ):
            xt = cpool.tile([128, D_SB, S, S], fp16)
            xt2 = cpool.tile([128, P2_SB, P2_IN_LINES, S], fp16)
            wt = cpool.tile([128, NG, 128], fp16)
            # DMA order = first-use order: tiny partial slab, then weights in
            # slices (matmul g only gates on its slice), then the main slab.
            # Dual z-shifted SBUF copies are built with two DMAs per plane
            # from the single DRAM slab: partitions 0..63 plane j <- plane j,
            # partitions 64..127 plane j <- plane j+1.
            nc.sync.dma_start(xt2[:64], x2_d[:, :P2_SB])
            nc.sync.dma_start(xt2[64:], x2_d[:, 1 : P2_SB + 1])
            # interleave weight slices (3 groups each) with main-slab plane
            # DMAs so neither starves during the partial block's compute
            wops = [
                lambda i=i: nc.sync.dma_start(wt[:, 3 * i : 3 * (i + 1)],
                                              w_d[:, 3 * i : 3 * (i + 1)])
                for i in range(25)
            ]
            xops = []
            for j in range(D_SB):
                xops.append(lambda j=j: nc.sync.dma_start(xt[:64, j], x_d[:, j]))
                xops.append(
                    lambda j=j: nc.sync.dma_start(xt[64:, j], x_d[:, j + 1])
                )
            while wops or xops:
                if wops:
                    wops.pop(0)()
                if xops:
                    xops.pop(0)()

            def do_chunk(src, z, ys, L, dst, zo):
                ps = ppool.tile([128, 9, SO], fp32)
                for g, (dy, dx, s) in enumerate(GROUPS):
                    rhs = src[:, z + s, ys + dy : ys + dy + L, dx : dx + SO]
                    nc.tensor.matmul(ps[:, :L, :], wt[:, g], rhs,
                                     start=(g == 0), stop=(g == NG - 1))
                ot = opool.tile([128, 9, SO], fp32)
                nc.vector.tensor_copy(ot[:, :L], ps[:, :L])
                dst2 = dst[zo : zo + 2, :, ys : ys + L, :].rearrange(
                    "z c l w -> (z c) l w"
                )
                nc.sync.dma_start(dst2, ot[:, :L])

            for _ in range(repeat):
                # partial first: its input lands quickly, hiding the main
                # slab's DMA behind ~21us of compute
                if partial:
                    for ys, L in CHUNKS2:
                        do_chunk(xt2, 0, ys, L, o2_d, 0)
                for p in range(n_pairs):
                    for ys, L in CHUNKS:
                        do_chunk(xt, 2 * p, ys, L, o_d, 2 * p)
    nc.compile()
    return nc


# ------------------------------------------------------------ entry point
LAST_RESULTS = None
LAST_NC = None
LAST_INMAPS = None


def kernel(x, weight, w_sc0, w_sc1):
    global LAST_RESULTS, LAST_NC, LAST_INMAPS
    x = np.asarray(x, dtype=np.float32)
    K = _fold_self_connection(
        _make_kernel(np.asarray(weight, dtype=np.float32)),
        np.asarray(w_sc0, dtype=np.float32),
        np.asarray(w_sc1, dtype=np.float32),
    )
    wk = _pack_weights(K)
    slabs = _pack_x(x)

    repeat = int(os.environ.get("KERNEL_REPEAT", "1"))
    nc = build_nc(repeat=repeat)
    in_maps = [{"x": slabs[c][0], "x2": slabs[c][1], "w": wk}
               for c in range(N_CORES)]
    res = run_bass_kernel_spmd(nc, in_maps, core_ids=list(range(N_CORES)))
    LAST_RESULTS, LAST_NC, LAST_INMAPS = res, nc, in_maps

    full = np.zeros((1, 64, SO, SO, SO), np.float32)
    for c in range(N_CORES):
        z0, p2, y2 = _core_assign(c)
        # device outputs are plane-major [z, c, l, w]
        full[0, :, z0 : z0 + D_OUT] = res.results[c]["out"].transpose(1, 0, 2, 3)
        full[0, :, p2 : p2 + 2, y2 : y2 + P2_LINES, :] = res.results[c][
            "out2"
        ].transpose(1, 0, 2, 3)
    return full


# revision 20
# speedup vs baseline: 1376.8692x; 1.0046x over previous
"""Trainium2 Bass kernel for nn_Convolution_1176821039998.

Equivariant (e3nn-style) 3D convolution, kernel 5x5x5, 64->64 channels, on a
[1,64,56,56,56] fp32 volume, plus a per-irrep self-connection on the cropped
volume.  Strategy:

Host side (tiny, fp32):
  - Build the dense conv kernel K[o,i,dz,dy,dx] from the TP weight exactly as
    the reference does, and fold the self-connection into the center tap.
  - Perfectly balanced z-shard across 8 cores with NO redundant compute:
    core c computes 6 "main" output planes 6c..6c+5 (planes 0..47) plus a
    13-line y-block of one of the remaining 2 plane-pairs (planes 48..51,
    block chosen by core index).  The partial block's input sub-volume is
    packed by the host into a small side tensor at FIXED local coordinates,
    so all cores run the identical SPMD program (3.25 plane-pairs each).
  - The kernel halo (4 planes) is handled by overlapping shards; no
    device-to-device exchange.

Device side (pair-packed matmuls using the full 128x128 PE array):
  - K = 128: 64 input channels x 2 adjacent input z-planes (dual z-shifted
    SBUF copies, built with two DMAs from one DRAM slab).
  - M = 128: 64 output channels x 2 adjacent OUTPUT z-planes.  A stream
    reading input plane z+s serves output plane z (taps dz=s low copy,
    s+1 high) and z+1 (dz=s-1 low, s high); streams s in {0,2,4} cover all
    5 z-taps of both planes -> 75 accumulating matmuls per plane-PAIR
    (25 (dy,dx) x 3), 2x fewer streamed columns than one-plane-at-a-time.
  - PSUM tile [128, L*52] fp32 per line-chunk; evict via VectorE, DMA out.

Numerics: inputs/weights fp16 (products exact in fp32, PSUM accumulates
fp32); measured max rel err vs fp32 reference ~4.5e-4.
"""

import os
import numpy as np

import concourse.bass as bass
import concourse.mybir as mybir
import concourse.tile as tile
from concourse import bacc
from concourse.bass_utils import run_bass_kernel_spmd

# ---------------------------------------------------------------- constants
SIZE = 5
MUL = 16
CROP = SIZE // 2
PW0 = np.float32((1.0 / 32.0) ** 0.5)
PW1 = np.float32((3.0 / 32.0) ** 0.5)
INV_SQRT3 = np.float32(3.0 ** -0.5)

N_CORES = 8
S = 56                                 # input spatial size
SO = 52                                # output spatial size
# main shard: 3 pairs = 6 output planes at z0 = 6c (covers 0..47)
N_PAIRS = 3
D_OUT = 6
D_DRAM = 10                            # main DRAM slab planes (6c..6c+9)
D_SB = 9                               # planes per SBUF copy (lo 0..8, hi 1..9)
# partial shard: planes 48..51 split into 8 (pair, 13-line block) quarters
P2_BASE = 48
P2_LINES = 13
P2_IN_LINES = P2_LINES + 4             # 17
P2_DRAM = 6                            # partial DRAM planes (P..P+5)
P2_SB = 5                              # partial SBUF planes per copy
CHUNKS = [(0, 9), (9, 9), (18, 9), (27, 9), (36, 9), (45, 7)]  # (y0, lines)
CHUNKS2 = [(0, 7), (7, 6)]             # partial 13-line block
# matmul group order: s-major so a chunk's first 25 matmuls only need input
# planes z..z+1 (and the first weight slices), letting compute start while
# later planes/weights are still in flight
S_STREAMS = (0, 2, 4)
GROUPS = [(dy, dx, s) for s in S_STREAMS for dy in range(5) for dx in range(5)]
NG = len(GROUPS)  # 75


def _core_assign(c):
    """(main z0, partial pair base, partial y0) for core c."""
    return 6 * c, P2_BASE + 2 * (c // 4), P2_LINES * (c % 4)


# ------------------------------------------------------- host-side weights
def _lattice_consts():
    r = np.linspace(-1.0, 1.0, SIZE, dtype=np.float32)
    lat = np.stack(np.meshgrid(r, r, r, indexing="ij"), axis=-1)
    d = np.linalg.norm(lat.astype(np.float64), axis=-1).astype(np.float32)
    values = np.linspace(0.0, 1.0, SIZE, dtype=np.float32)
    step = values[1] - values[0]
    diff = (d[..., None] - values) / step

    def sus(t):
        return np.where(t > 0, np.exp(-1.0 / np.where(t > 0, t, 1.0)), 0.0).astype(
            np.float32
        )

    emb = np.float32(1.14136) * np.float32(np.e ** 2) * sus(diff + 1.0) * sus(1.0 - diff)
    n = lat / np.maximum(d, 1e-12)[..., None]
    sh0 = np.ones_like(d)
    sh1 = np.float32(3.0 ** 0.5) * n
    return emb.astype(np.float32), sh0, sh1.astype(np.float32)


def _make_kernel(weight):
    """[5,1024] -> conv kernel [out=64, in=64, 5,5,5] fp32 (mirrors reference)."""
    emb, sh0, sh1 = _lattice_consts()
    w = emb @ weight
    Ssp = w.shape[:3]
    blk = MUL * MUL
    w1, w2, w3, w4 = [
        w[..., i * blk : (i + 1) * blk].reshape(*Ssp, MUL, MUL) for i in range(4)
    ]
    k_ss = PW0 * w1 * sh0[..., None, None]
    k_sv = PW1 * INV_SQRT3 * np.einsum("...uw,...k->...uwk", w2, sh1)
    k_vs = PW0 * INV_SQRT3 * np.einsum("...uw,...i->...uiw", w4, sh1)
    eye3 = np.eye(3, dtype=w.dtype)
    k_vv = (
        PW1
        * INV_SQRT3
        * (w3 * sh0[..., None, None])[..., :, None, :, None]
        * eye3[None, None, None, None, :, None, :]
    )
    top = np.concatenate([k_ss, k_sv.reshape(*Ssp, MUL, 3 * MUL)], axis=-1)
    bot = np.concatenate(
        [k_vs.reshape(*Ssp, 3 * MUL, MUL), k_vv.reshape(*Ssp, 3 * MUL, 3 * MUL)],
        axis=-1,
    )
    kernel = np.concatenate([top, bot], axis=-2)  # [5,5,5,in,out]
    return np.ascontiguousarray(np.transpose(kernel, (4, 3, 0, 1, 2)))


def _fold_self_connection(K, w_sc0, w_sc1):
    """Add the cropped e3nn Linear self-connection into the center tap."""
    inv = np.float32(1.0 / MUL ** 0.5)
    sc = np.zeros((64, 64), np.float32)
    sc[:MUL, :MUL] = w_sc0.T * inv  # sc[out w, in u] = w_sc0[u, w]
    for wo in range(MUL):
        for u in range(MUL):
            for k in range(3):
                sc[MUL + 3 * wo + k, MUL + 3 * u + k] += w_sc1[u, wo] * inv
    K = K.copy()
    K[:, :, CROP, CROP, CROP] += sc
    return K


def _pack_weights(K, dtype=np.float16):
    """[64,64,5,5,5] -> lhsT tiles [128, NG, 128] in GROUPS order.

    lhsT rows: 64 in-channels x {low copy (plane z+s), high copy (z+s+1)}.
    lhsT cols: 64 out-channels x {out plane z, out plane z+1}.
    Block (row half r, col half m) holds tap dz = s + r - m (zero if outside
    0..4)."""
    wk = np.zeros((128, NG, 128), np.float32)
    for g, (dy, dx, s) in enumerate(GROUPS):
        for r in range(2):
            for m in range(2):
                dz = s + r - m
                if 0 <= dz < 5:
                    wk[64 * r : 64 * r + 64, g, 64 * m : 64 * m + 64] = K[
                        :, :, dz, dy, dx
                    ].T
    return np.ascontiguousarray(wk.astype(dtype))


def _pack_x(x, dtype=np.float16):
    """x [1,64,56,56,56] -> per-core (main slab [64,10,56,56],
    partial slab [64,6,17,56])."""
    slabs = []
    for c in range(N_CORES):
        z0, p2, y2 = _core_assign(c)
        xa = np.ascontiguousarray(x[0, :, z0 : z0 + D_DRAM].astype(dtype))
        xp = np.ascontiguousarray(
            x[0, :, p2 : p2 + P2_DRAM, y2 : y2 + P2_IN_LINES].astype(dtype)
        )
        slabs.append((xa, xp))
    return slabs


# ------------------------------------------------------- device program
def build_nc(n_pairs=N_PAIRS, partial=True, repeat=1):
    fp16 = mybir.dt.float16
    fp32 = mybir.dt.float32
    nc = bacc.Bacc("TRN2", target_bir_lowering=False, debug=False,
                   num_devices=N_CORES)
    x_d = nc.dram_tensor("x", [64, D_DRAM, S, S], fp16, kind="ExternalInput").ap()
    x2_d = nc.dram_tensor("x2", [64, P2_DRAM, P2_IN_LINES, S], fp16,
                          kind="ExternalInput").ap()
    w_d = nc.dram_tensor("w", [128, NG, 128], fp16, kind="ExternalInput").ap()
    # outputs are plane-major so one DMA can write both planes of a pair:
    # SBUF partitions (z c) = plane-half * 64 + channel
    o_d = nc.dram_tensor("out", [2 * n_pairs, 64, SO, SO], fp32,
                         kind="ExternalOutput").ap()
    o2_d = nc.dram_tensor("out2", [2, 64, P2_LINES, SO], fp32,
                          kind="ExternalOutput").ap()

    with tile.TileContext(nc) as tc:
        with (
            tc.tile_pool(name="const", bufs=1) as cpool,
            tc.tile_pool(name="outp", bufs=3) as opool,
            tc.tile_pool(name="psum", bufs=8, space="PSUM") as ppool,
        ):
            xt = cpool.tile([128, D_SB, S, S], fp16)
            xt2 = cpool.tile([128, P2_SB, P2_IN_LINES, S], fp16)
            wt = cpool.tile([128, NG, 128], fp16)
            # DMA order = first-use order: tiny partial slab, then weights in
            # slices (matmul g only gates on its slice), then the main slab.
            # Dual z-shifted SBUF copies are built with two DMAs per plane
            # from the single DRAM slab: partitions 0..63 plane j <- plane j,
            # partitions 64..127 plane j <- plane j+1.
            # partial slab in two slices per half: planes 0..1 first (gates
            # the s=0 matmuls), then 2..4
            nc.sync.dma_start(xt2[:64, :2], x2_d[:, :2])
            nc.sync.dma_start(xt2[64:, :2], x2_d[:, 1:3])
            nc.sync.dma_start(xt2[:64, 2:], x2_d[:, 2:P2_SB])
            nc.sync.dma_start(xt2[64:, 2:], x2_d[:, 3 : P2_SB + 1])
            # interleave weight slices (3 groups each) with main-slab plane
            # DMAs so neither starves during the partial block's compute
            wops = [
                lambda i=i: nc.sync.dma_start(wt[:, 3 * i : 3 * (i + 1)],
                                              w_d[:, 3 * i : 3 * (i + 1)])
                for i in range(25)
            ]
            xops = []
            for j in range(D_SB):
                xops.append(lambda j=j: nc.sync.dma_start(xt[:64, j], x_d[:, j]))
                xops.append(
                    lambda j=j: nc.sync.dma_start(xt[64:, j], x_d[:, j + 1])
                )
            while wops or xops:
                if wops:
                    wops.pop(0)()
                if xops:
                    xops.pop(0)()

            def do_chunk(src, z, ys, L, dst, zo):
                ps = ppool.tile([128, 9, SO], fp32)
                for g, (dy, dx, s) in enumerate(GROUPS):
                    rhs = src[:, z + s, ys + dy : ys + dy + L, dx : dx + SO]
                    nc.tensor.matmul(ps[:, :L, :], wt[:, g], rhs,
                                     start=(g == 0), stop=(g == NG - 1))
                ot = opool.tile([128, 9, SO], fp32)
                nc.vector.tensor_copy(ot[:, :L], ps[:, :L])
                dst2 = dst[zo : zo + 2, :, ys : ys + L, :].rearrange(
                    "z c l w -> (z c) l w"
                )
                nc.sync.dma_start(dst2, ot[:, :L])

            for _ in range(repeat):
                # partial first: its input lands quickly, hiding the main
                # slab's DMA behind ~21us of compute
                if partial:
                    for ys, L in CHUNKS2:
                        do_chunk(xt2, 0, ys, L, o2_d, 0)
                for p in range(n_pairs):
                    for ys, L in CHUNKS:
                        do_chunk(xt, 2 * p, ys, L, o_d, 2 * p)
    nc.compile()
    return nc


# ------------------------------------------------------------ entry point
LAST_RESULTS = None
LAST_NC = None
LAST_INMAPS = None


def kernel(x, weight, w_sc0, w_sc1):
    global LAST_RESULTS, LAST_NC, LAST_INMAPS
    x = np.asarray(x, dtype=np.float32)
    K = _fold_self_connection(
        _make_kernel(np.asarray(weight, dtype=np.float32)),
        np.asarray(w_sc0, dtype=np.float32),
        np.asarray(w_sc1, dtype=np.float32),
    )
    wk = _pack_weights(K)
    slabs = _pack_x(x)

    repeat = int(os.environ.get("KERNEL_REPEAT", "1"))
    nc = build_nc(repeat=repeat)
    in_maps = [{"x": slabs[c][0], "x2": slabs[c][1], "w": wk}
               for c in range(N_CORES)]
    res = run_bass_kernel_spmd(nc, in_maps, core_ids=list(range(N_CORES)))
    LAST_RESULTS, LAST_NC, LAST_INMAPS = res, nc, in_maps

    full = np.zeros((1, 64, SO, SO, SO), np.float32)
    for c in range(N_CORES):
        z0, p2, y2 = _core_assign(c)
        # device outputs are plane-major [z, c, l, w]
        full[0, :, z0 : z0 + D_OUT] = res.results[c]["out"].transpose(1, 0, 2, 3)
        full[0, :, p2 : p2 + 2, y2 : y2 + P2_LINES, :] = res.results[c][
            "out2"
        ].transpose(1, 0, 2, 3)
    return full
